# revision 14
# baseline (speedup 1.0000x reference)
"""CPC loss kernel for Trainium2 (Bass/Tile), data-parallel over batch on 8 NeuronCores.

Math: the reference's exp/log cancel exactly; the loss is a masked sum of dot
products: step_loss_k = -1/(B(T-i)) * sum_{b,t<lim} sum_e mctx[b,t,e,k]*bmn[b,t+i,e]
with i = k+1, lim = min(seq_len[b], T-i), bmn = base - sum_n neg_samples.

Device layout: e (=128) on partitions, t on the free dim. Per (row, k): DVE
multiplies mctxT[e, t]*bmn[e, t+k+1] (the shift is a free-dim AP offset), then a
TensorE matmul with a one-hot-column stationary reduces over partitions, landing
column sums in PSUM partition row r*K+k (zeros accumulate elsewhere). A final
reduce_sum over t gives per-(row,k) scalars; host applies the -1/(B(T-i)) scales.

v2 over baseline:
 - Rows sorted by seq_len desc, assigned (slot s, core c) = rank s*8+c, so all
   cores share slot widths Ls = seq-derived (JIT-specialized program). DMA and
   compute are sliced to Ls (~75% of T on average).
 - Host zeroes mctx tails (t >= lim) so no mask tensor / mask-multiply needed;
   PSUM columns beyond a slot's Ls stay zero via the one-hot scheme (slot 0 is
   widest and initializes the full PSUM width with start=True).
 - The shift-by-1 copy of bmn (for odd-shift 4B alignment) runs on ScalarE
   instead of an SBUF->SBUF DMA, removing ~2.1MB of SDMA traffic.
 - DMA queues balanced: m_lo on sync, m_hi on scalar, bmn on gpsimd.
 - Slot 0 is panel-split (2x512 cols per half) so compute starts earlier.
"""

import numpy as np

B, T, E, K, NNEG = 64, 1024, 128, 8, 64
NCORES = 8
NSLOT = B // NCORES          # 8 slots (one row per slot per core)
KH = K // 2
KORDER = [1, 3, 5, 7, 0, 2, 4, 6]   # lo half: shifts 2,4,6,8 ; hi half: 1,3,5,7
PAD = 16                     # bmn width pad so every shifted window is in-bounds

MODE = "fp16"
_CACHE = {}
TRACE = False
TRACE_KWARGS = {}
LAST_RESULTS = None


def _build(slot_lens):
    from contextlib import ExitStack
    import concourse.bass as bass
    import concourse.bacc as bacc
    import concourse.tile as tile
    import concourse.mybir as mybir

    f32 = mybir.dt.float32
    f16 = mybir.dt.float16
    L0 = slot_lens[0]
    NR = NSLOT * K

    nc = bacc.Bacc(
        "TRN2",
        target_bir_lowering=False,
        debug=False,
        enable_asserts=False,
        num_devices=NCORES,
    )
    m_in = []
    bmn_in = []
    P0 = slot_lens[0] // 512  # slot 0 stored panel-major for contiguous DMAs
    for s, Ls in enumerate(slot_lens):
        shape = [E, P0, K, 512] if s == 0 else [E, K, Ls]
        m_in.append(nc.dram_tensor(f"m{s}", shape, f16, kind="ExternalInput").ap())
        bmn_in.append(
            nc.dram_tensor(f"bmn{s}", [E, Ls + PAD], f16, kind="ExternalInput").ap())
    wide_in = nc.dram_tensor("wide", [E, NR - 1], f16, kind="ExternalInput").ap()
    s_out = nc.dram_tensor("S", [NR, 1], f32, kind="ExternalOutput").ap()

    NG = NR // 2  # 32 psum rows per group (slots 0-3 / slots 4-7)
    with tile.TileContext(nc) as tc, ExitStack() as ctx:
        m_pool = ctx.enter_context(tc.tile_pool(name="m", bufs=3))
        bmn_pool = ctx.enter_context(tc.tile_pool(name="bmn", bufs=2))
        prod_pool = ctx.enter_context(tc.tile_pool(name="prod", bufs=2))
        misc_pool = ctx.enter_context(tc.tile_pool(name="misc", bufs=1))
        psum_pool = ctx.enter_context(tc.tile_pool(name="psum", bufs=1, space="PSUM"))

        # wide[:, NG-1] = 1, else 0. wide[:, NG-1-row : 2*NG-1-row] is a
        # ones-column at position `row`: the matmul lands the column sum of the
        # moving operand in PSUM partition `row` of its group, zeros elsewhere.
        wide = misc_pool.tile([E, 2 * NG - 1], f16)
        nc.gpsimd.dma_start(wide[:], wide_in[:, :])
        ps_a = psum_pool.tile([NG, L0], f32, tag="ps_a")
        ps_b = psum_pool.tile([NG, L0], f32, tag="ps_b")
        ps_g = [ps_a, ps_b]
        s_tile = misc_pool.tile([NR, 1], f32)

        def panels_of(s, Ls):
            if s == 0:
                return [(0, 512)] + ([(512, Ls)] if Ls > 512 else [])
            return [(0, Ls)]

        # greedy DMA queue balance by measured per-queue rates; gpsimd
        # (slow SWDGE startup) is kept off the early slots.
        qload = {"sync": 0.0, "scalar": 0.0, "gpsimd": 0.0}
        qrate = {"sync": 165.0, "scalar": 150.0, "gpsimd": 112.0}
        qeng = {"sync": nc.sync, "scalar": nc.scalar, "gpsimd": nc.gpsimd}

        def pick_queue(nbytes, allow_gp):
            cands = ["sync", "scalar"] + (["gpsimd"] if allow_gp else [])
            q = min(cands, key=lambda n: (qload[n] + nbytes) / qrate[n])
            qload[q] += nbytes
            return qeng[q]

        WBMAX = L0 + PAD
        for s, Ls in enumerate(slot_lens):
            WB = Ls + PAD
            ps = ps_g[s // 4]
            bmn = bmn_pool.tile([E, WBMAX], f16, tag="bmn")
            if s == 0:
                qload["sync"] += E * WB * 2
                nc.sync.dma_start(bmn[:, 0:WB], bmn_in[s])
            else:
                pick_queue(E * WB * 2, 2 <= s <= 5).dma_start(
                    bmn[:, 0:WB], bmn_in[s])

            panels = panels_of(s, Ls)
            for half in ("lo", "hi"):
                khi = KH if half == "lo" else K
                if s == 0:
                    eng = nc.scalar if half == "lo" else nc.sync
                    qload["scalar" if half == "lo" else "sync"] += \
                        E * KH * Ls * 2
                else:
                    eng = pick_queue(E * KH * Ls * 2, 2 <= s <= 5)
                # hi planes are host-shifted right by one so their bmn window
                # offsets are even (korig 2j -> offset 2j); lo offsets 2j+2.
                woff = 2 if half == "lo" else 0
                for pi, (c0, c1) in enumerate(panels):
                    tsuf = f"{half}_{pi}_s0" if s == 0 else f"{half}_full"
                    mp = misc_pool if s == 0 else m_pool
                    pp = misc_pool if s == 0 else prod_pool
                    MW = 512 if s == 0 else L0
                    W = c1 - c0
                    m_t = mp.tile([E, KH, MW], f16, tag=f"m_{tsuf}")
                    # flat contiguous SBUF view for the DMA (few descriptors)
                    m_flat = bass.AP(m_t[:].tensor, 0, [[KH * MW, E], [1, KH * W]])
                    if s == 0:
                        eng.dma_start(m_flat, m_in[s][:, pi, khi - KH:khi, :])
                    else:
                        eng.dma_start(m_flat, m_in[s][:, khi - KH:khi, 0:Ls])
                    m_v = bass.AP(m_t[:].tensor, 0, [[KH * MW, E], [W, KH], [1, W]])
                    prod = pp.tile([E, KH, MW], f16, tag=f"prod_{tsuf}")
                    prod_v = bass.AP(prod[:].tensor, 0,
                                     [[KH * MW, E], [W, KH], [1, W]])
                    src = bass.AP(bmn[:].tensor, woff + c0,
                                  [[WBMAX, E], [2, KH], [1, W]])
                    nc.vector.tensor_mul(prod_v, m_v, src)
                    for j in range(KH):
                        row = (s % 4) * K + (j if half == "lo" else KH + j)
                        oh = wide[:, NG - 1 - row:2 * NG - 1 - row]
                        for (d0, d1) in ([(c0, c1)] if s == 0
                                         else [(0, 512), (512, Ls)]):
                            if d1 <= d0:
                                continue
                            first = s % 4 == 0 and row == 0
                            last = s % 4 == 3 and row == NG - 1 and d1 == Ls
                            rhs = bass.AP(prod[:].tensor, j * W + d0 - c0,
                                          [[KH * MW, E], [1, d1 - d0]])
                            nc.tensor.matmul(
                                ps[:, d0:d1], lhsT=oh, rhs=rhs,
                                start=first, stop=last,
                                skip_group_check=True,
                            )
            if s == 3:
                # group-a finisher overlaps slots 4-7 (separate PSUM tile);
                # out-DMA on gpsimd so its completion hides under later work
                nc.vector.reduce_sum(s_tile[0:NG, :], ps_g[0][:, 0:L0],
                                     axis=mybir.AxisListType.X)
                nc.gpsimd.dma_start(s_out[0:NG, :], s_tile[0:NG, :])

        L4 = slot_lens[4]
        nc.vector.reduce_sum(s_tile[NG:NR, :], ps_g[1][:, 0:L4],
                             axis=mybir.AxisListType.X)
        nc.scalar.dma_start(s_out[NG:NR, :], s_tile[NG:NR, :])

    nc.compile()
    return nc


def kernel(base_emb, mapped_ctx, seq_lens, neg_ids):
    global LAST_RESULTS
    from concourse import bass_utils

    base = np.ascontiguousarray(np.asarray(base_emb, dtype=np.float32))
    mctx = np.asarray(mapped_ctx, dtype=np.float32)
    seq = np.asarray(seq_lens, dtype=np.int32)
    nids = np.asarray(neg_ids, dtype=np.int32)

    # Host prep: per-batch negative gather (per sharding hint), bmn = base - negsum
    neg_sum = base.reshape(B * T, E)[nids].sum(axis=1)             # [B, E]
    bmn = (base - neg_sum[:, None, :]).astype(np.float16)          # [B, T, E]

    # Row -> (slot, core) assignment: sort by needed width desc; slot s takes
    # ranks [8s, 8s+8), one per core. All cores share slot widths.
    lim = np.minimum(seq[:, None], (T - 1 - np.arange(K))[None, :])  # [B, K] per korig
    need = lim.max(axis=1)                                           # [B]
    order = np.argsort(-need, kind="stable")                         # rank -> b
    slot_lens = []
    for s in range(NSLOT):
        group = order[s * NCORES:(s + 1) * NCORES]
        Ls = int(need[group].max()) + 1   # +1: hi planes are shifted right
        Ls = min(T, max(512, -(-Ls // 64) * 64))
        if s == 0:
            Ls = min(T, -(-Ls // 512) * 512)
        slot_lens.append(Ls)
    slot_lens = tuple(slot_lens)
    P0 = slot_lens[0] // 512

    NG = NSLOT * K // 2
    wide = np.zeros((E, 2 * NG - 1), np.float16)
    wide[:, NG - 1] = 1.0

    key = ("nc", MODE, slot_lens)
    if key not in _CACHE:
        _CACHE[key] = _build(slot_lens)
    nc = _CACHE[key]

    in_maps = [{"wide": wide} for _ in range(NCORES)]
    for s in range(NSLOT):
        Ls = slot_lens[s]
        for c in range(NCORES):
            b = int(order[s * NCORES + c])
            mT = mctx[b].transpose(1, 2, 0)[:, KORDER, :]          # [E, K, T]
            out = np.zeros((E, K, Ls), np.float16)
            for j, korig in enumerate(KORDER):
                l = int(lim[b, korig])
                if j < KH:
                    out[:, j, :l] = mT[:, j, :l]
                else:
                    # hi planes shifted right by one -> even bmn offsets
                    out[:, j, 1:l + 1] = mT[:, j, :l]
            if s == 0:
                out = np.ascontiguousarray(
                    out.reshape(E, K, P0, 512).transpose(0, 2, 1, 3))
            bT = np.zeros((E, Ls + PAD), np.float16)
            w = min(T, Ls + PAD)
            bT[:, :w] = bmn[b, :w].T
            in_maps[c][f"m{s}"] = out
            in_maps[c][f"bmn{s}"] = bT

    res = bass_utils.run_bass_kernel_spmd(
        nc, in_maps, core_ids=list(range(NCORES)), trace=TRACE, **TRACE_KWARGS
    )
    LAST_RESULTS = res

    loss = 0.0
    for c in range(NCORES):
        S = res.results[c]["S"].reshape(NSLOT, K)                  # [slot, korder-idx]
        for s in range(NSLOT):
            for j, korig in enumerate(KORDER):
                loss += -S[s, j] / (B * (T - 1 - korig))
    loss /= K
    return np.float32(loss)


# revision 16
# speedup vs baseline: 1.0942x; 1.0942x over previous
"""CPC loss kernel for Trainium2 (Bass/Tile), data-parallel over batch on 8 NeuronCores.

Math: the reference's exp/log cancel exactly; the loss is a masked sum of dot
products: step_loss_k = -1/(B(T-i)) * sum_{b,t<lim} sum_e mctx[b,t,e,k]*bmn[b,t+i,e]
with i = k+1, lim = min(seq_len[b], T-i), bmn = base - sum_n neg_samples.

Device layout: e (=128) on partitions, t on the free dim. Per (row, k): DVE
multiplies mctxT[e, t]*bmn[e, t+k+1] (the shift is a free-dim AP offset), then a
TensorE matmul with a one-hot-column stationary reduces over partitions, landing
column sums in PSUM partition row r*K+k (zeros accumulate elsewhere). A final
reduce_sum over t gives per-(row,k) scalars; host applies the -1/(B(T-i)) scales.

v2 over baseline:
 - Rows sorted by seq_len desc, assigned (slot s, core c) = rank s*8+c, so all
   cores share slot widths Ls = seq-derived (JIT-specialized program). DMA and
   compute are sliced to Ls (~75% of T on average).
 - Host zeroes mctx tails (t >= lim) so no mask tensor / mask-multiply needed;
   PSUM columns beyond a slot's Ls stay zero via the one-hot scheme (slot 0 is
   widest and initializes the full PSUM width with start=True).
 - The shift-by-1 copy of bmn (for odd-shift 4B alignment) runs on ScalarE
   instead of an SBUF->SBUF DMA, removing ~2.1MB of SDMA traffic.
 - DMA queues balanced: m_lo on sync, m_hi on scalar, bmn on gpsimd.
 - Slot 0 is panel-split (2x512 cols per half) so compute starts earlier.
"""

import numpy as np

B, T, E, K, NNEG = 64, 1024, 128, 8, 64
NCORES = 8
NSLOT = B // NCORES          # 8 slots (one row per slot per core)
KH = K // 2
KORDER = [1, 3, 5, 7, 0, 2, 4, 6]   # lo half: shifts 2,4,6,8 ; hi half: 1,3,5,7
PAD = 16                     # bmn width pad so every shifted window is in-bounds

MODE = "fp16"
_CACHE = {}
TRACE = False
TRACE_KWARGS = {}
LAST_RESULTS = None


def _build(slot_lens):
    from contextlib import ExitStack
    import concourse.bass as bass
    import concourse.bacc as bacc
    import concourse.tile as tile
    import concourse.mybir as mybir

    f32 = mybir.dt.float32
    f16 = mybir.dt.float16
    L0 = slot_lens[0]
    NR = NSLOT * K

    nc = bacc.Bacc(
        "TRN2",
        target_bir_lowering=False,
        debug=False,
        enable_asserts=False,
        num_devices=NCORES,
    )
    m_in = []
    bmn_in = []
    P0 = slot_lens[0] // 512  # slot 0 stored panel-major for contiguous DMAs
    for s, Ls in enumerate(slot_lens):
        shape = [E, P0, K, 512] if s == 0 else [E, K, Ls]
        m_in.append(nc.dram_tensor(f"m{s}", shape, f16, kind="ExternalInput").ap())
        bmn_in.append(
            nc.dram_tensor(f"bmn{s}", [E, Ls + PAD], f16, kind="ExternalInput").ap())
    wide_in = nc.dram_tensor("wide", [E, NR - 1], f16, kind="ExternalInput").ap()
    s_out = nc.dram_tensor("S", [NR, 1], f32, kind="ExternalOutput").ap()

    NG = NR // 2  # 32 psum rows per group (slots 0-3 / slots 4-7)
    with tile.TileContext(nc) as tc, ExitStack() as ctx:
        m_pool = ctx.enter_context(tc.tile_pool(name="m", bufs=3))
        bmn_pool = ctx.enter_context(tc.tile_pool(name="bmn", bufs=2))
        prod_pool = ctx.enter_context(tc.tile_pool(name="prod", bufs=2))
        misc_pool = ctx.enter_context(tc.tile_pool(name="misc", bufs=1))
        psum_pool = ctx.enter_context(tc.tile_pool(name="psum", bufs=1, space="PSUM"))

        # wide[:, NG-1] = 1, else 0. wide[:, NG-1-row : 2*NG-1-row] is a
        # ones-column at position `row`: the matmul lands the column sum of the
        # moving operand in PSUM partition `row` of its group, zeros elsewhere.
        wide = misc_pool.tile([E, 2 * NG - 1], f16)
        nc.gpsimd.dma_start(wide[:], wide_in[:, :])
        ps_a = psum_pool.tile([NG, L0], f32, tag="ps_a")
        ps_b = psum_pool.tile([NG, L0], f32, tag="ps_b")
        ps_g = [ps_a, ps_b]
        s_tile = misc_pool.tile([NR, 1], f32)

        def panels_of(s, Ls):
            if s == 0:
                return [(0, 512)] + ([(512, Ls)] if Ls > 512 else [])
            return [(0, Ls)]

        # greedy DMA queue balance by measured per-queue rates; gpsimd
        # (slow SWDGE path) only gets small bmn transfers, mid-kernel.
        qload = {"sync": 0.0, "scalar": 0.0, "gpsimd": 0.0}
        qrate = {"sync": 165.0, "scalar": 160.0, "gpsimd": 40.0}
        qeng = {"sync": nc.sync, "scalar": nc.scalar, "gpsimd": nc.gpsimd}

        def pick_queue(nbytes, allow_gp):
            cands = ["sync", "scalar"] + (["gpsimd"] if allow_gp else [])
            q = min(cands, key=lambda n: (qload[n] + nbytes) / qrate[n])
            qload[q] += nbytes
            return qeng[q]

        WBMAX = L0 + PAD
        for s, Ls in enumerate(slot_lens):
            WB = Ls + PAD
            ps = ps_g[s // 4]
            bmn = bmn_pool.tile([E, WBMAX], f16, tag="bmn")
            if s == 0:
                qload["sync"] += E * WB * 2
                nc.sync.dma_start(bmn[:, 0:WB], bmn_in[s])
            else:
                pick_queue(E * WB * 2, 2 <= s <= 5).dma_start(
                    bmn[:, 0:WB], bmn_in[s])

            panels = panels_of(s, Ls)
            for half in ("lo", "hi"):
                khi = KH if half == "lo" else K
                if s == 0:
                    eng = nc.scalar if half == "lo" else nc.sync
                    qload["scalar" if half == "lo" else "sync"] += \
                        E * KH * Ls * 2
                else:
                    eng = pick_queue(E * KH * Ls * 2, False)
                # hi planes are host-shifted right by one so their bmn window
                # offsets are even (korig 2j -> offset 2j); lo offsets 2j+2.
                woff = 2 if half == "lo" else 0
                for pi, (c0, c1) in enumerate(panels):
                    tsuf = f"{half}_{pi}_s0" if s == 0 else f"{half}_full"
                    mp = misc_pool if s == 0 else m_pool
                    pp = misc_pool if s == 0 else prod_pool
                    MW = 512 if s == 0 else L0
                    W = c1 - c0
                    m_t = mp.tile([E, KH, MW], f16, tag=f"m_{tsuf}")
                    if s == 0:
                        eng.dma_start(m_t[:, :, 0:W],
                                      m_in[s][:, pi, khi - KH:khi, :])
                    else:
                        eng.dma_start(m_t[:, :, 0:W],
                                      m_in[s][:, khi - KH:khi, 0:Ls])
                    prod = pp.tile([E, KH, MW], f16, tag=f"prod_{tsuf}")
                    src = bass.AP(bmn[:].tensor, woff + c0,
                                  [[WBMAX, E], [2, KH], [1, W]])
                    nc.vector.tensor_mul(
                        prod[:, :, 0:W], m_t[:, :, 0:W], src)
                    for j in range(KH):
                        row = (s % 4) * K + (j if half == "lo" else KH + j)
                        oh = wide[:, NG - 1 - row:2 * NG - 1 - row]
                        for (d0, d1) in ([(c0, c1)] if s == 0
                                         else [(0, 512), (512, Ls)]):
                            if d1 <= d0:
                                continue
                            first = s % 4 == 0 and row == 0
                            last = s % 4 == 3 and row == NG - 1 and d1 == Ls
                            nc.tensor.matmul(
                                ps[:, d0:d1], lhsT=oh,
                                rhs=prod[:, j, d0 - c0:d1 - c0],
                                start=first, stop=last,
                                skip_group_check=True,
                            )
            if s == 3:
                # group-a finisher overlaps slots 4-7 (separate PSUM tile);
                # out-DMA on gpsimd so its completion hides under later work
                nc.vector.reduce_sum(s_tile[0:NG, :], ps_g[0][:, 0:L0],
                                     axis=mybir.AxisListType.X)
                nc.gpsimd.dma_start(s_out[0:NG, :], s_tile[0:NG, :])

        L4 = slot_lens[4]
        nc.vector.reduce_sum(s_tile[NG:NR, :], ps_g[1][:, 0:L4],
                             axis=mybir.AxisListType.X)
        nc.scalar.dma_start(s_out[NG:NR, :], s_tile[NG:NR, :])

    nc.compile()
    return nc


def kernel(base_emb, mapped_ctx, seq_lens, neg_ids):
    global LAST_RESULTS
    from concourse import bass_utils

    base = np.ascontiguousarray(np.asarray(base_emb, dtype=np.float32))
    mctx = np.asarray(mapped_ctx, dtype=np.float32)
    seq = np.asarray(seq_lens, dtype=np.int32)
    nids = np.asarray(neg_ids, dtype=np.int32)

    # Host prep: per-batch negative gather (per sharding hint), bmn = base - negsum
    neg_sum = base.reshape(B * T, E)[nids].sum(axis=1)             # [B, E]
    bmn = (base - neg_sum[:, None, :]).astype(np.float16)          # [B, T, E]

    # Row -> (slot, core) assignment: sort by needed width desc; slot s takes
    # ranks [8s, 8s+8), one per core. All cores share slot widths.
    lim = np.minimum(seq[:, None], (T - 1 - np.arange(K))[None, :])  # [B, K] per korig
    need = lim.max(axis=1)                                           # [B]
    order = np.argsort(-need, kind="stable")                         # rank -> b
    slot_lens = []
    for s in range(NSLOT):
        group = order[s * NCORES:(s + 1) * NCORES]
        Ls = int(need[group].max()) + 1   # +1: hi planes are shifted right
        Ls = min(T, max(512, -(-Ls // 64) * 64))
        if s == 0:
            Ls = min(T, -(-Ls // 512) * 512)
        slot_lens.append(Ls)
    slot_lens = tuple(slot_lens)
    P0 = slot_lens[0] // 512

    NG = NSLOT * K // 2
    wide = np.zeros((E, 2 * NG - 1), np.float16)
    wide[:, NG - 1] = 1.0

    key = ("nc", MODE, slot_lens)
    if key not in _CACHE:
        _CACHE[key] = _build(slot_lens)
    nc = _CACHE[key]

    in_maps = [{"wide": wide} for _ in range(NCORES)]
    for s in range(NSLOT):
        Ls = slot_lens[s]
        for c in range(NCORES):
            b = int(order[s * NCORES + c])
            mT = mctx[b].transpose(1, 2, 0)[:, KORDER, :]          # [E, K, T]
            out = np.zeros((E, K, Ls), np.float16)
            for j, korig in enumerate(KORDER):
                l = int(lim[b, korig])
                if j < KH:
                    out[:, j, :l] = mT[:, j, :l]
                else:
                    # hi planes shifted right by one -> even bmn offsets
                    out[:, j, 1:l + 1] = mT[:, j, :l]
            if s == 0:
                out = np.ascontiguousarray(
                    out.reshape(E, K, P0, 512).transpose(0, 2, 1, 3))
            bT = np.zeros((E, Ls + PAD), np.float16)
            w = min(T, Ls + PAD)
            bT[:, :w] = bmn[b, :w].T
            in_maps[c][f"m{s}"] = out
            in_maps[c][f"bmn{s}"] = bT

    res = bass_utils.run_bass_kernel_spmd(
        nc, in_maps, core_ids=list(range(NCORES)), trace=TRACE, **TRACE_KWARGS
    )
    LAST_RESULTS = res

    loss = 0.0
    for c in range(NCORES):
        S = res.results[c]["S"].reshape(NSLOT, K)                  # [slot, korder-idx]
        for s in range(NSLOT):
            for j, korig in enumerate(KORDER):
                loss += -S[s, j] / (B * (T - 1 - korig))
    loss /= K
    return np.float32(loss)
